# revision 27
# baseline (speedup 1.0000x reference)
"""GAT (graph attention network) Bass kernel for 8 trn2 NeuronCores.

Strategy (row-sharding): core k owns query rows [k*512, (k+1)*512).
 - Every core computes Wh = x @ W[h] for ALL nodes (replicated, cheap on PE)
   in [node-on-partition, feature] layout; s1 = x@(W a1) and s2 = x@(W a2)
   come from one thin matmul against a packed [wa1|wa2] weight block.
 - Hidden attention per head, transposed layout [keys j on partitions,
   own rows i free]: z = leaky(s1[i]+s2[j]+(-100)*!mask) in one
   scalar_tensor_tensor; leaky+exp run on ScalarE (Prelu+Exp share one
   ACT table set) over 8-block slabs to amortize the ~352-cycle ACT
   fixed cost; out_head.T = [Wh|1].T @ P accumulated in PSUM gives both
   att@Wh and softmax denominators.
 - Normalize + elu -> h kept transposed as lhsT for Wh_o = h @ W_out.
   AllGather of Wh_o [512,16] -> [4096,16] is the only cross-core
   exchange; the output attention layer then runs the same way,
   followed by elu + log_softmax.
"""

import sys

sys.path.insert(0, "/opt/trn_rl_repo")

import numpy as np
import ml_dtypes

import concourse.bass as bass
import concourse.bacc as bacc
import concourse.tile as tile
from concourse import mybir
from concourse.bass_utils import run_bass_kernel_spmd
from concourse.masks import make_identity

F32 = mybir.dt.float32
BF16 = mybir.dt.bfloat16
BF = ml_dtypes.bfloat16
ADD = mybir.AluOpType.add
MULT = mybir.AluOpType.mult
MAX = mybir.AluOpType.max
AF = mybir.ActivationFunctionType

# problem shape (hardcoded per spec)
N = 4096
F_IN = 512
O = 64
H = 8
C = 16
N_CORES = 8
NEG = -100.0  # additive mask offset; exp(leaky(-100+e)) <= ~1e-8
ALPHA = 0.2

# knobs
DVE_LEAKY = False  # leaky on VectorE (sim-safe) instead of ScalarE Prelu
GROUP = 8          # j-blocks per activation slab

KF = F_IN // 128   # f_in k-tiles


def _build_nc(n_cores=N_CORES, n=N):
    NB = n // 128          # node blocks (keys)
    OWN = n // n_cores     # own query rows per core
    OB = OWN // 128        # own row blocks
    NG = NB // GROUP       # slabs per attention pass
    nc = bacc.Bacc("TRN2", target_bir_lowering=False, debug=False,
                   num_devices=n_cores)

    # per-core external inputs (host-packed, see _pack_inputs)
    d_xT = nc.dram_tensor("xT", [128, KF * n], BF16, kind="ExternalInput")
    d_xo = nc.dram_tensor("xo", [128, KF * OWN], BF16, kind="ExternalInput")
    d_w64 = nc.dram_tensor("w64", [128, H * KF * O], BF16, kind="ExternalInput")
    d_waA = nc.dram_tensor("waA", [128, KF * 2 * H], BF16, kind="ExternalInput")
    d_wo1 = nc.dram_tensor("wo1r", [128, KF * 128], BF16, kind="ExternalInput")
    d_nm = nc.dram_tensor("nmT", [128, NB * OWN], BF16, kind="ExternalInput")
    d_wot = nc.dram_tensor("wot", [128, KF * C], BF16, kind="ExternalInput")
    d_a2o = nc.dram_tensor("a2o", [128, C], F32, kind="ExternalInput")
    d_out = nc.dram_tensor("out", [OWN, C], F32, kind="ExternalOutput")

    with tile.TileContext(nc) as tc:
        with (
            tc.tile_pool(name="dram", bufs=1, space="DRAM") as dram,
            tc.tile_pool(name="const", bufs=1) as const,
            tc.tile_pool(name="work", bufs=2) as work,
            tc.tile_pool(name="small", bufs=3) as small,
            tc.tile_pool(name="psA", bufs=3, space="PSUM") as psA,
            tc.tile_pool(name="psH", bufs=2, space="PSUM") as psH,
            tc.tile_pool(name="psM", bufs=2, space="PSUM") as psM,
        ):
            # ---- load inputs ----
            xT = const.tile([128, KF * n], BF16)
            nc.sync.dma_start(out=xT, in_=d_xT[:])
            xo = const.tile([128, KF * OWN], BF16)
            nc.sync.dma_start(out=xo, in_=d_xo[:])
            w64 = const.tile([128, H * KF * O], BF16)
            nc.sync.dma_start(out=w64, in_=d_w64[:])
            waA = const.tile([128, KF * 2 * H], BF16)
            nc.sync.dma_start(out=waA, in_=d_waA[:])
            wo1 = const.tile([128, KF * 128], BF16)
            nc.sync.dma_start(out=wo1, in_=d_wo1[:])
            nm = const.tile([128, NB * OWN], BF16)
            nc.sync.dma_start(out=nm, in_=d_nm[:])
            wot = const.tile([128, KF * C], BF16)
            nc.sync.dma_start(out=wot, in_=d_wot[:])
            a2o = const.tile([128, C], F32)
            nc.sync.dma_start(out=a2o, in_=d_a2o[:])

            ident = const.tile([128, 128], F32)
            make_identity(nc, ident[:])
            identb = const.tile([16, 16], BF16)
            make_identity(nc, identb[:])

            wht = [const.tile([128, NB * 65], BF16, tag=f"wh{h}", name=f"wh{h}")
                   for h in range(H)]
            for h in range(H):
                # ones column at j*65+64 (strided memset)
                nc.vector.memset(
                    wht[h][:].rearrange("p (b w) -> p b w", w=65)[:, :, 64:65], 1.0)
            hT_all = const.tile([128, KF * OWN], BF16)

            # ---- phase S: s1/s2 rows via thin matmuls ----
            # s1 for own rows -> broadcast to all partitions per head
            ps1 = psM.tile([16, OWN], F32, tag="mm", name="ps1")
            for k in range(KF):
                nc.tensor.matmul(ps1[:], waA[:, k * 16: (k + 1) * 16],
                                 xo[:, k * OWN: (k + 1) * OWN],
                                 start=(k == 0), stop=(k == KF - 1))
            s1T = const.tile([16, OWN], BF16)
            nc.vector.tensor_copy(s1T[:], ps1[:])
            # broadcast row h to 128 partitions via a DRAM bounce
            s1d = dram.tile([8, OWN], BF16)
            nc.sync.dma_start(out=s1d[:], in_=s1T[8:16, :])
            s1b = [const.tile([128, OWN], BF16, tag=f"s1b{h}", name=f"s1b{h}")
                   for h in range(H)]
            for h in range(H):
                nc.sync.dma_start(out=s1b[h][:],
                                  in_=s1d[h: h + 1, :].to_broadcast([128, OWN]))
            # s2 rows for all nodes; s2f split per slab-group so the first
            # slab's stt is not gated on the whole transpose sweep
            s2T = const.tile([8, n], BF16)
            for ch in range(n // 512):
                pss = psM.tile([16, 512], F32, tag="mm", name=f"s12_{ch}")
                for k in range(KF):
                    nc.tensor.matmul(
                        pss[:], waA[:, k * 16: (k + 1) * 16],
                        xT[:, k * n + ch * 512: k * n + ch * 512 + 512],
                        start=(k == 0), stop=(k == KF - 1))
                nc.vector.tensor_copy(s2T[:, ch * 512: (ch + 1) * 512],
                                      pss[0:8, :])
            # s2fg[g][p, q*8+h] = s2[h][(g*GROUP+q)*128+p]
            NGG = NB // GROUP
            s2fg = [const.tile([128, GROUP * 8], F32, tag=f"s2f{g}",
                               name=f"s2f{g}") for g in range(NGG)]
            for jb in range(NB):
                pst = psM.tile([128, 8], BF16, tag="s2t", name=f"s2t_{jb}",
                               bufs=1)
                nc.tensor.transpose(pst[:], s2T[:, jb * 128: (jb + 1) * 128],
                                    identb[0:8, 0:8])
                nc.vector.tensor_copy(
                    s2fg[jb // GROUP][:, (jb % GROUP) * 8: (jb % GROUP) * 8 + 8],
                    pst[:])

            # ---- phase A helper: Wh for one head, one 4-block chunk;
            # 4 node-blocks per PSUM bank as sequential accumulation groups,
            # one batched cast. Emission is interleaved into phase B so the
            # in-order engine queues pipeline A(h+1) under B(h). ----
            def emit_wh_chunk(h, nb4):
                ps = psA.tile([128, 4 * O], F32, tag="whp",
                              name=f"whp_{h}_{nb4}")
                for sub in range(4):
                    nb = nb4 * 4 + sub
                    for k in range(KF):
                        nc.tensor.matmul(
                            ps[:, sub * O: (sub + 1) * O],
                            xT[:, k * n + nb * 128: k * n + nb * 128 + 128],
                            w64[:, (h * KF + k) * O: (h * KF + k) * O + O],
                            start=(k == 0), stop=(k == KF - 1))
                dst = (wht[h][:, nb4 * 4 * 65: (nb4 * 4 + 4) * 65]
                       .rearrange("p (b w) -> p b w", w=65)[:, :, 0:O])
                src = ps[:].rearrange("p (b w) -> p b w", w=O)
                nc.vector.tensor_copy(dst, src)

            NB4 = NB // 4
            for nb4 in range(NB4):
                emit_wh_chunk(0, nb4)

            # ---- attention slab helper (hidden + output layers) ----
            def attention(s2col, s1bt, lhsT_tile, lhsw, m_rows, psacc, tagp,
                          pre_slab=None):
                for g in range(NG):
                    if pre_slab is not None:
                        pre_slab(g)
                    zs = work.tile([128, GROUP * OWN], BF16, tag="z",
                                   name=f"z{tagp}_{g}")
                    for q in range(GROUP):
                        jb = g * GROUP + q
                        nc.vector.scalar_tensor_tensor(
                            zs[:, q * OWN: (q + 1) * OWN],
                            nm[:, jb * OWN: (jb + 1) * OWN],
                            s2col(jb), s1bt[:], ADD, ADD)
                    us = work.tile([128, GROUP * OWN], BF16, tag="p",
                                   name=f"u{tagp}_{g}")
                    if DVE_LEAKY:
                        nc.vector.tensor_scalar(us[:], zs[:], ALPHA, None, MULT)
                        nc.vector.tensor_tensor(us[:], us[:], zs[:], MAX)
                    else:
                        nc.scalar.activation(us[:], zs[:], AF.Prelu, alpha=ALPHA)
                    nc.scalar.activation(zs[:], us[:], AF.Exp)
                    for q in range(GROUP):
                        jb = g * GROUP + q
                        nc.tensor.matmul(
                            psacc[0: m_rows, :],
                            lhsT_tile[:, jb * lhsw: jb * lhsw + m_rows],
                            zs[:, q * OWN: (q + 1) * OWN],
                            start=(jb == 0), stop=(jb == NB - 1))

            # ---- phase B: hidden attention ----
            rstage = const.tile([65, OWN], F32)

            def finalize_head(h, ph):
                nc.vector.tensor_copy(rstage[64:65, :], ph[64:65, :])
                rcp = small.tile([65, OWN], F32, tag="rcp", name=f"rcp{h}")
                nc.vector.reciprocal(rcp[64:65, :], rstage[64:65, :])
                rd = dram.tile([1, OWN], F32, name=f"rd{h}")
                nc.sync.dma_start(out=rd[:], in_=rcp[64:65, :])
                rb = small.tile([64, OWN], F32, tag="rb", name=f"rb{h}")
                nc.sync.dma_start(out=rb[:],
                                  in_=rd[0:1, :].to_broadcast([64, OWN]))
                tn = small.tile([64, OWN], F32, tag="tn", name=f"tn{h}")
                nc.vector.tensor_tensor(tn[:], ph[0:64, :], rb[:], MULT)
                m0 = small.tile([64, OWN], F32, tag="m0", name=f"m0{h}")
                nc.vector.tensor_scalar(m0[:], tn[:], 0.0, None,
                                        mybir.AluOpType.min)
                g_ = small.tile([64, OWN], F32, tag="g", name=f"g{h}")
                nc.scalar.activation(g_[:], m0[:], AF.Exp)
                slot = hT_all[(h % 2) * 64: (h % 2) * 64 + 64,
                              (h // 2) * OWN: (h // 2) * OWN + OWN]
                nc.vector.scalar_tensor_tensor(slot, g_[:], -1.0, tn[:], ADD, MAX)

            for h in range(H):
                ph = psH.tile([65, OWN], F32, tag="ph", name=f"ph{h}")

                def pre_slab(g, h=h):
                    # emit next head's Wh chunks under this head's slabs
                    if h + 1 < H:
                        per = (NB4 + NG - 1) // NG
                        for j in range(g * per, min((g + 1) * per, NB4)):
                            emit_wh_chunk(h + 1, j)

                attention(lambda jb, h=h: s2fg[jb // GROUP][:, (jb % GROUP) * 8 + h: (jb % GROUP) * 8 + h + 1],
                          s1b[h], wht[h], 65, 65, ph, f"h{h}",
                          pre_slab=pre_slab)
                finalize_head(h, ph)

            # ---- phase C: Wh_o own rows (+s2o col) -> AllGather ----
            # s1ob depends only on hT_all: emit before the collective
            s1ob = const.tile([128, OWN], BF16)
            ps1o = psM.tile([128, OWN], F32, tag="mm", name="ps1o")
            for c in range(KF):
                nc.tensor.matmul(ps1o[:], wo1[:, c * 128: (c + 1) * 128],
                                 hT_all[:, c * OWN: (c + 1) * OWN],
                                 start=(c == 0), stop=(c == KF - 1))
            nc.vector.tensor_copy(s1ob[:], ps1o[:])

            whoown = const.tile([128, OB * 17], F32)
            for ib in range(OB):
                pw = psM.tile([128, OWN], F32, tag="mm", name=f"pw{ib}")
                for c in range(KF):
                    nc.tensor.matmul(
                        pw[:, 0:C],
                        hT_all[:, c * OWN + ib * 128: c * OWN + ib * 128 + 128],
                        wot[:, c * C: (c + 1) * C],
                        start=(c == 0), stop=(c == KF - 1))
                nc.vector.tensor_copy(whoown[:, ib * 17: ib * 17 + C],
                                      pw[:, 0:C])
                tmp = small.tile([128, C], F32, tag="s2tmp", name=f"s2o{ib}")
                nc.vector.scalar_tensor_tensor(
                    tmp[:], pw[:, 0:C], 1.0, a2o[:], MULT, MULT,
                    accum_out=whoown[:, ib * 17 + 16: ib * 17 + 17])
            cc_in = dram.tile([128, OB * 17], F32)
            cc_out = dram.tile([n_cores * 128, OB * 17], F32,
                               addr_space="Shared" if n_cores > 1 else "Local")
            nc.gpsimd.dma_start(out=cc_in[:], in_=whoown[:])
            if n_cores > 1:
                nc.gpsimd.collective_compute(
                    "AllGather", mybir.AluOpType.bypass,
                    replica_groups=[list(range(n_cores))],
                    ins=[cc_in.opt()], outs=[cc_out.opt()])
            else:
                nc.gpsimd.dma_start(out=cc_out[:], in_=cc_in[:])
            # whoall[p, jb*17+c] = [Wh_o | s2o][jb*128+p, c], jb = g*OB + ib
            whoall = const.tile([128, NB * 17], F32)
            nc.gpsimd.dma_start(
                out=whoall[:],
                in_=cc_out[:].rearrange("(g p) f -> p g f", p=128))

            who17 = const.tile([128, NB * 17], BF16)
            nc.vector.memset(
                who17[:].rearrange("p (b w) -> p b w", w=17)[:, :, 16:17], 1.0)
            nc.vector.tensor_copy(
                who17[:].rearrange("p (b w) -> p b w", w=17)[:, :, 0:C],
                whoall[:].rearrange("p (b w) -> p b w", w=17)[:, :, 0:C])

            # ---- phase D: output attention ----
            po = psM.tile([128, OWN], F32, tag="mm", name="po")
            attention(lambda jb: whoall[:, jb * 17 + 16: jb * 17 + 17],
                      s1ob, who17, 17, 17, po, "o")

            # ---- phase E: transpose, normalize, elu, log_softmax, store ----
            osb = const.tile([17, OWN], F32)
            nc.vector.tensor_copy(osb[:], po[0:17, :])
            e1all = const.tile([128, OB * C], F32)
            sall = const.tile([128, OB], F32)
            final = const.tile([128, OB * C], F32)
            for tt in range(OB):
                ptr = psM.tile([128, OWN], F32, tag="mm", name=f"ptr{tt}")
                nc.tensor.transpose(ptr[:, 0:17],
                                    osb[0:17, tt * 128: (tt + 1) * 128],
                                    ident[0:17, 0:17])
                rr = small.tile([128, 1], F32, tag="rr", name=f"rr{tt}")
                nc.vector.reciprocal(rr[:], ptr[:, 16:17])
                t1 = small.tile([128, C], F32, tag="t1", name=f"t1{tt}")
                nc.vector.tensor_scalar(t1[:], ptr[:, 0:C], rr[:], None, MULT)
                m1 = small.tile([128, C], F32, tag="m1", name=f"m1{tt}")
                nc.vector.tensor_scalar(m1[:], t1[:], 0.0, None,
                                        mybir.AluOpType.min)
                g1 = small.tile([128, C], F32, tag="g1", name=f"g1{tt}")
                nc.scalar.activation(g1[:], m1[:], AF.Exp)
                nc.vector.scalar_tensor_tensor(
                    e1all[:, tt * C: (tt + 1) * C], g1[:], -1.0, t1[:], ADD, MAX)
                ex = small.tile([128, C], F32, tag="ex", name=f"ex{tt}")
                nc.scalar.activation(ex[:], e1all[:, tt * C: (tt + 1) * C],
                                     AF.Exp, accum_out=sall[:, tt: tt + 1])
            for tt in range(OB):
                lns = small.tile([128, 1], F32, tag="lns", name=f"lns{tt}")
                nc.scalar.activation(lns[:], sall[:, tt: tt + 1], AF.Ln)
                nc.vector.tensor_scalar(final[:, tt * C: (tt + 1) * C],
                                        e1all[:, tt * C: (tt + 1) * C],
                                        lns[:], None, mybir.AluOpType.subtract)
            nc.sync.dma_start(
                out=d_out[:].rearrange("(b p) c -> p b c", p=128),
                in_=final[:])

    nc.compile()
    return nc


def _pack_inputs(x, adj, W, a, W_out, a_out, n_cores=N_CORES):
    """Host-side shard + layout packing. Returns list of per-core in_maps."""
    n, f_in = x.shape
    OWN = n // n_cores
    NB = n // 128
    xf = np.asarray(x, np.float32)
    Wf = np.asarray(W, np.float32)
    af = np.asarray(a, np.float32)
    Wof = np.asarray(W_out, np.float32)
    aof = np.asarray(a_out, np.float32)

    # xT[p, k*n + m] = x[m, 128k+p]
    xT = xf.T.reshape(KF, 128, n).transpose(1, 0, 2).reshape(128, KF * n)
    xT = xT.astype(BF)
    w64 = (Wf.reshape(H, KF, 128, O).transpose(2, 0, 1, 3)
           .reshape(128, H * KF * O).astype(BF))
    wa1 = np.einsum("hfo,ho->hf", Wf, af[:, :O])  # [H, F]
    wa2 = np.einsum("hfo,ho->hf", Wf, af[:, O:])
    # waA[p, k*16 + m]: m<8 -> wa2[m], else wa1[m-8]
    waA = np.concatenate([wa2, wa1], axis=0)  # [16, F]
    waA = waA.T.reshape(KF, 128, 16).transpose(1, 0, 2).reshape(128, KF * 16)
    waA = waA.astype(BF)
    wo1 = Wof @ aof[:C]  # [F]
    wo1r = np.broadcast_to(
        wo1.reshape(KF, 128).T[:, :, None], (128, KF, 128)
    ).reshape(128, KF * 128).astype(BF)
    wot = (Wof.reshape(KF, 128, C).transpose(1, 0, 2)
           .reshape(128, KF * C).astype(BF))
    a2o = np.broadcast_to(aof[C:], (128, C)).astype(np.float32).copy()

    in_maps = []
    for core in range(n_cores):
        rows = slice(core * OWN, (core + 1) * OWN)
        xo = (xf[rows].T.reshape(KF, 128, OWN).transpose(1, 0, 2)
              .reshape(128, KF * OWN).astype(BF))
        nmT = np.where(adj[rows].T > 0, np.float32(0), np.float32(NEG))
        nmT = (nmT.reshape(NB, 128, OWN).transpose(1, 0, 2)
               .reshape(128, NB * OWN).astype(BF))
        in_maps.append({
            "xT": xT, "xo": xo, "w64": w64, "waA": waA, "wo1r": wo1r,
            "nmT": nmT, "wot": wot, "a2o": a2o,
        })
    return in_maps


_NC_CACHE = {}


def _get_nc(n_cores=N_CORES, n=N):
    key = (n_cores, n)
    if key not in _NC_CACHE:
        _NC_CACHE[key] = _build_nc(n_cores, n)
    return _NC_CACHE[key]


def kernel(x, adj, W, a, W_out, a_out):
    nc = _get_nc()
    in_maps = _pack_inputs(x, adj, W, a, W_out, a_out)
    res = run_bass_kernel_spmd(nc, in_maps, list(range(N_CORES)))
    out = np.concatenate([res.results[c]["out"] for c in range(N_CORES)], axis=0)
    return out.astype(np.float32)


# revision 28
# speedup vs baseline: 1.0139x; 1.0139x over previous
"""GAT (graph attention network) Bass kernel for 8 trn2 NeuronCores.

Strategy (row-sharding): core k owns query rows [k*512, (k+1)*512).
 - Every core computes Wh = x @ W[h] for ALL nodes (replicated, cheap on PE)
   in [node-on-partition, feature] layout; s1 = x@(W a1) and s2 = x@(W a2)
   come from one thin matmul against a packed [wa1|wa2] weight block.
 - Hidden attention per head, transposed layout [keys j on partitions,
   own rows i free]: z = leaky(s1[i]+s2[j]+(-100)*!mask) in one
   scalar_tensor_tensor; leaky+exp run on ScalarE (Prelu+Exp share one
   ACT table set) over 8-block slabs to amortize the ~352-cycle ACT
   fixed cost; out_head.T = [Wh|1].T @ P accumulated in PSUM gives both
   att@Wh and softmax denominators.
 - Normalize + elu -> h kept transposed as lhsT for Wh_o = h @ W_out.
   AllGather of Wh_o [512,16] -> [4096,16] is the only cross-core
   exchange; the output attention layer then runs the same way,
   followed by elu + log_softmax.
"""

import sys

sys.path.insert(0, "/opt/trn_rl_repo")

import numpy as np
import ml_dtypes

import concourse.bass as bass
import concourse.bacc as bacc
import concourse.tile as tile
from concourse import mybir
from concourse.bass_utils import run_bass_kernel_spmd
from concourse.masks import make_identity

F32 = mybir.dt.float32
BF16 = mybir.dt.bfloat16
BF = ml_dtypes.bfloat16
ADD = mybir.AluOpType.add
MULT = mybir.AluOpType.mult
MAX = mybir.AluOpType.max
AF = mybir.ActivationFunctionType

# problem shape (hardcoded per spec)
N = 4096
F_IN = 512
O = 64
H = 8
C = 16
N_CORES = 8
NEG = -100.0  # additive mask offset; exp(leaky(-100+e)) <= ~1e-8
ALPHA = 0.2

# knobs
DVE_LEAKY = False  # leaky on VectorE (sim-safe) instead of ScalarE Prelu
GROUP = 8          # j-blocks per activation slab

KF = F_IN // 128   # f_in k-tiles


def _build_nc(n_cores=N_CORES, n=N):
    NB = n // 128          # node blocks (keys)
    OWN = n // n_cores     # own query rows per core
    OB = OWN // 128        # own row blocks
    NG = NB // GROUP       # slabs per attention pass
    nc = bacc.Bacc("TRN2", target_bir_lowering=False, debug=False,
                   num_devices=n_cores)

    # per-core external inputs (host-packed, see _pack_inputs)
    d_xT = nc.dram_tensor("xT", [128, KF * n], BF16, kind="ExternalInput")
    d_xo = nc.dram_tensor("xo", [128, KF * OWN], BF16, kind="ExternalInput")
    d_w64 = nc.dram_tensor("w64", [128, H * KF * O], BF16, kind="ExternalInput")
    d_waA = nc.dram_tensor("waA", [128, KF * 2 * H], BF16, kind="ExternalInput")
    d_wo1 = nc.dram_tensor("wo1r", [128, KF * 128], BF16, kind="ExternalInput")
    d_nm = nc.dram_tensor("nmT", [128, NB * OWN], BF16, kind="ExternalInput")
    d_wot = nc.dram_tensor("wot", [128, KF * C], BF16, kind="ExternalInput")
    d_a2o = nc.dram_tensor("a2o", [128, C], F32, kind="ExternalInput")
    d_out = nc.dram_tensor("out", [OWN, C], F32, kind="ExternalOutput")

    with tile.TileContext(nc) as tc:
        with (
            tc.tile_pool(name="dram", bufs=1, space="DRAM") as dram,
            tc.tile_pool(name="const", bufs=1) as const,
            tc.tile_pool(name="work", bufs=2) as work,
            tc.tile_pool(name="small", bufs=3) as small,
            tc.tile_pool(name="psA", bufs=3, space="PSUM") as psA,
            tc.tile_pool(name="psH", bufs=2, space="PSUM") as psH,
            tc.tile_pool(name="psM", bufs=2, space="PSUM") as psM,
        ):
            # ---- load inputs ----
            xT = const.tile([128, KF * n], BF16)
            nc.sync.dma_start(out=xT, in_=d_xT[:])
            xo = const.tile([128, KF * OWN], BF16)
            nc.sync.dma_start(out=xo, in_=d_xo[:])
            w64 = const.tile([128, H * KF * O], BF16)
            nc.sync.dma_start(out=w64, in_=d_w64[:])
            waA = const.tile([128, KF * 2 * H], BF16)
            nc.sync.dma_start(out=waA, in_=d_waA[:])
            wo1 = const.tile([128, KF * 128], BF16)
            nc.sync.dma_start(out=wo1, in_=d_wo1[:])
            nm = const.tile([128, NB * OWN], BF16)
            nc.sync.dma_start(out=nm, in_=d_nm[:])
            wot = const.tile([128, KF * C], BF16)
            nc.sync.dma_start(out=wot, in_=d_wot[:])
            a2o = const.tile([128, C], F32)
            nc.sync.dma_start(out=a2o, in_=d_a2o[:])

            ident = const.tile([128, 128], F32)
            make_identity(nc, ident[:])
            identb = const.tile([16, 16], BF16)
            make_identity(nc, identb[:])

            wht = [const.tile([128, NB * 65], BF16, tag=f"wh{h}", name=f"wh{h}")
                   for h in range(H)]
            for h in range(H):
                # ones column at j*65+64 (strided memset)
                nc.vector.memset(
                    wht[h][:].rearrange("p (b w) -> p b w", w=65)[:, :, 64:65], 1.0)
            hT_all = const.tile([128, KF * OWN], BF16)

            # ---- phase S: s1/s2 rows via thin matmuls ----
            # s1 for own rows -> broadcast to all partitions per head
            ps1 = psM.tile([16, OWN], F32, tag="mm", name="ps1")
            for k in range(KF):
                nc.tensor.matmul(ps1[:], waA[:, k * 16: (k + 1) * 16],
                                 xo[:, k * OWN: (k + 1) * OWN],
                                 start=(k == 0), stop=(k == KF - 1))
            s1T = const.tile([16, OWN], BF16)
            nc.vector.tensor_copy(s1T[:], ps1[:])
            # broadcast row h to 128 partitions via a DRAM bounce
            s1d = dram.tile([8, OWN], BF16)
            nc.sync.dma_start(out=s1d[:], in_=s1T[8:16, :])
            s1b = [const.tile([128, OWN], BF16, tag=f"s1b{h}", name=f"s1b{h}")
                   for h in range(H)]
            for h in range(H):
                nc.sync.dma_start(out=s1b[h][:],
                                  in_=s1d[h: h + 1, :].to_broadcast([128, OWN]))
            # s2 rows for all nodes; s2f split per slab-group so the first
            # slab's stt is not gated on the whole transpose sweep
            s2T = const.tile([8, n], BF16)
            for ch in range(n // 512):
                pss = psM.tile([16, 512], F32, tag="mm", name=f"s12_{ch}")
                for k in range(KF):
                    nc.tensor.matmul(
                        pss[:], waA[:, k * 16: (k + 1) * 16],
                        xT[:, k * n + ch * 512: k * n + ch * 512 + 512],
                        start=(k == 0), stop=(k == KF - 1))
                nc.vector.tensor_copy(s2T[:, ch * 512: (ch + 1) * 512],
                                      pss[0:8, :])
            # s2fg[g][p, q*8+h] = s2[h][(g*GROUP+q)*128+p]
            NGG = NB // GROUP
            s2fg = [const.tile([128, GROUP * 8], F32, tag=f"s2f{g}",
                               name=f"s2f{g}") for g in range(NGG)]
            for jb in range(NB):
                pst = psM.tile([128, 8], BF16, tag="s2t", name=f"s2t_{jb}",
                               bufs=1)
                nc.tensor.transpose(pst[:], s2T[:, jb * 128: (jb + 1) * 128],
                                    identb[0:8, 0:8])
                nc.vector.tensor_copy(
                    s2fg[jb // GROUP][:, (jb % GROUP) * 8: (jb % GROUP) * 8 + 8],
                    pst[:])

            # ---- phase A helper: Wh for one head, one 4-block chunk;
            # 4 node-blocks per PSUM bank as sequential accumulation groups,
            # one batched cast. Emission is interleaved into phase B so the
            # in-order engine queues pipeline A(h+1) under B(h). ----
            def emit_wh_chunk(h, nb4):
                ps = psA.tile([128, 4 * O], F32, tag="whp",
                              name=f"whp_{h}_{nb4}")
                for sub in range(4):
                    nb = nb4 * 4 + sub
                    for k in range(KF):
                        nc.tensor.matmul(
                            ps[:, sub * O: (sub + 1) * O],
                            xT[:, k * n + nb * 128: k * n + nb * 128 + 128],
                            w64[:, (h * KF + k) * O: (h * KF + k) * O + O],
                            start=(k == 0), stop=(k == KF - 1))
                dst = (wht[h][:, nb4 * 4 * 65: (nb4 * 4 + 4) * 65]
                       .rearrange("p (b w) -> p b w", w=65)[:, :, 0:O])
                src = ps[:].rearrange("p (b w) -> p b w", w=O)
                nc.vector.tensor_copy(dst, src)

            NB4 = NB // 4

            # ---- attention slab helper (hidden + output layers) ----
            def attention(s2col, s1bt, lhsT_tile, lhsw, m_rows, psacc, tagp,
                          pre_slab=None):
                for g in range(NG):
                    if pre_slab is not None:
                        pre_slab(g)
                    zs = work.tile([128, GROUP * OWN], BF16, tag="z",
                                   name=f"z{tagp}_{g}")
                    for q in range(GROUP):
                        jb = g * GROUP + q
                        nc.vector.scalar_tensor_tensor(
                            zs[:, q * OWN: (q + 1) * OWN],
                            nm[:, jb * OWN: (jb + 1) * OWN],
                            s2col(jb), s1bt[:], ADD, ADD)
                    us = work.tile([128, GROUP * OWN], BF16, tag="p",
                                   name=f"u{tagp}_{g}")
                    if DVE_LEAKY:
                        nc.vector.tensor_scalar(us[:], zs[:], ALPHA, None, MULT)
                        nc.vector.tensor_tensor(us[:], us[:], zs[:], MAX)
                    else:
                        nc.scalar.activation(us[:], zs[:], AF.Prelu, alpha=ALPHA)
                    nc.scalar.activation(zs[:], us[:], AF.Exp)
                    for q in range(GROUP):
                        jb = g * GROUP + q
                        nc.tensor.matmul(
                            psacc[0: m_rows, :],
                            lhsT_tile[:, jb * lhsw: jb * lhsw + m_rows],
                            zs[:, q * OWN: (q + 1) * OWN],
                            start=(jb == 0), stop=(jb == NB - 1))

            # ---- phase B: hidden attention ----
            rstage = const.tile([65, OWN], F32)

            def finalize_head(h, ph):
                nc.vector.tensor_copy(rstage[64:65, :], ph[64:65, :])
                rcp = small.tile([65, OWN], F32, tag="rcp", name=f"rcp{h}")
                nc.vector.reciprocal(rcp[64:65, :], rstage[64:65, :])
                rd = dram.tile([1, OWN], F32, name=f"rd{h}")
                nc.sync.dma_start(out=rd[:], in_=rcp[64:65, :])
                rb = small.tile([64, OWN], F32, tag="rb", name=f"rb{h}")
                nc.sync.dma_start(out=rb[:],
                                  in_=rd[0:1, :].to_broadcast([64, OWN]))
                tn = small.tile([64, OWN], F32, tag="tn", name=f"tn{h}")
                nc.vector.tensor_tensor(tn[:], ph[0:64, :], rb[:], MULT)
                m0 = small.tile([64, OWN], F32, tag="m0", name=f"m0{h}")
                nc.vector.tensor_scalar(m0[:], tn[:], 0.0, None,
                                        mybir.AluOpType.min)
                g_ = small.tile([64, OWN], F32, tag="g", name=f"g{h}")
                nc.scalar.activation(g_[:], m0[:], AF.Exp)
                slot = hT_all[(h % 2) * 64: (h % 2) * 64 + 64,
                              (h // 2) * OWN: (h // 2) * OWN + OWN]
                nc.vector.scalar_tensor_tensor(slot, g_[:], -1.0, tn[:], ADD, MAX)

            per = (NB4 + NG - 1) // NG
            prev = [None]
            for h in range(H):
                ph = psH.tile([65, OWN], F32, tag="ph", name=f"ph{h}")

                def pre_slab(g, h=h, ph=ph):
                    if h == 0:
                        # head 0: emit its own Wh chunks just in time
                        for j in range(g * per, min((g + 1) * per, NB4)):
                            emit_wh_chunk(0, j)
                    if h + 1 < H:
                        # emit next head's Wh chunks under this head's slabs
                        for j in range(g * per, min((g + 1) * per, NB4)):
                            emit_wh_chunk(h + 1, j)
                    if g == 1 and prev[0] is not None:
                        # previous head's normalize/elu, off the critical path
                        finalize_head(h - 1, prev[0])

                attention(lambda jb, h=h: s2fg[jb // GROUP][:, (jb % GROUP) * 8 + h: (jb % GROUP) * 8 + h + 1],
                          s1b[h], wht[h], 65, 65, ph, f"h{h}",
                          pre_slab=pre_slab)
                prev[0] = ph
            finalize_head(H - 1, prev[0])

            # ---- phase C: Wh_o own rows (+s2o col) -> AllGather ----
            # s1ob depends only on hT_all: emit before the collective
            s1ob = const.tile([128, OWN], BF16)
            ps1o = psM.tile([128, OWN], F32, tag="mm", name="ps1o")
            for c in range(KF):
                nc.tensor.matmul(ps1o[:], wo1[:, c * 128: (c + 1) * 128],
                                 hT_all[:, c * OWN: (c + 1) * OWN],
                                 start=(c == 0), stop=(c == KF - 1))
            nc.vector.tensor_copy(s1ob[:], ps1o[:])

            whoown = const.tile([128, OB * 17], F32)
            for ib in range(OB):
                pw = psM.tile([128, OWN], F32, tag="mm", name=f"pw{ib}")
                for c in range(KF):
                    nc.tensor.matmul(
                        pw[:, 0:C],
                        hT_all[:, c * OWN + ib * 128: c * OWN + ib * 128 + 128],
                        wot[:, c * C: (c + 1) * C],
                        start=(c == 0), stop=(c == KF - 1))
                nc.vector.tensor_copy(whoown[:, ib * 17: ib * 17 + C],
                                      pw[:, 0:C])
                tmp = small.tile([128, C], F32, tag="s2tmp", name=f"s2o{ib}")
                nc.vector.scalar_tensor_tensor(
                    tmp[:], pw[:, 0:C], 1.0, a2o[:], MULT, MULT,
                    accum_out=whoown[:, ib * 17 + 16: ib * 17 + 17])
            cc_in = dram.tile([128, OB * 17], F32)
            cc_out = dram.tile([n_cores * 128, OB * 17], F32,
                               addr_space="Shared" if n_cores > 1 else "Local")
            nc.gpsimd.dma_start(out=cc_in[:], in_=whoown[:])
            if n_cores > 1:
                nc.gpsimd.collective_compute(
                    "AllGather", mybir.AluOpType.bypass,
                    replica_groups=[list(range(n_cores))],
                    ins=[cc_in.opt()], outs=[cc_out.opt()])
            else:
                nc.gpsimd.dma_start(out=cc_out[:], in_=cc_in[:])
            # whoall[p, jb*17+c] = [Wh_o | s2o][jb*128+p, c], jb = g*OB + ib
            whoall = const.tile([128, NB * 17], F32)
            nc.gpsimd.dma_start(
                out=whoall[:],
                in_=cc_out[:].rearrange("(g p) f -> p g f", p=128))

            who17 = const.tile([128, NB * 17], BF16)
            nc.vector.memset(
                who17[:].rearrange("p (b w) -> p b w", w=17)[:, :, 16:17], 1.0)
            nc.vector.tensor_copy(
                who17[:].rearrange("p (b w) -> p b w", w=17)[:, :, 0:C],
                whoall[:].rearrange("p (b w) -> p b w", w=17)[:, :, 0:C])

            # ---- phase D: output attention ----
            po = psM.tile([128, OWN], F32, tag="mm", name="po")
            attention(lambda jb: whoall[:, jb * 17 + 16: jb * 17 + 17],
                      s1ob, who17, 17, 17, po, "o")

            # ---- phase E: transpose, normalize, elu, log_softmax, store ----
            osb = const.tile([17, OWN], F32)
            nc.vector.tensor_copy(osb[:], po[0:17, :])
            e1all = const.tile([128, OB * C], F32)
            sall = const.tile([128, OB], F32)
            final = const.tile([128, OB * C], F32)
            for tt in range(OB):
                ptr = psM.tile([128, OWN], F32, tag="mm", name=f"ptr{tt}")
                nc.tensor.transpose(ptr[:, 0:17],
                                    osb[0:17, tt * 128: (tt + 1) * 128],
                                    ident[0:17, 0:17])
                rr = small.tile([128, 1], F32, tag="rr", name=f"rr{tt}")
                nc.vector.reciprocal(rr[:], ptr[:, 16:17])
                t1 = small.tile([128, C], F32, tag="t1", name=f"t1{tt}")
                nc.vector.tensor_scalar(t1[:], ptr[:, 0:C], rr[:], None, MULT)
                m1 = small.tile([128, C], F32, tag="m1", name=f"m1{tt}")
                nc.vector.tensor_scalar(m1[:], t1[:], 0.0, None,
                                        mybir.AluOpType.min)
                g1 = small.tile([128, C], F32, tag="g1", name=f"g1{tt}")
                nc.scalar.activation(g1[:], m1[:], AF.Exp)
                nc.vector.scalar_tensor_tensor(
                    e1all[:, tt * C: (tt + 1) * C], g1[:], -1.0, t1[:], ADD, MAX)
                ex = small.tile([128, C], F32, tag="ex", name=f"ex{tt}")
                nc.scalar.activation(ex[:], e1all[:, tt * C: (tt + 1) * C],
                                     AF.Exp, accum_out=sall[:, tt: tt + 1])
            for tt in range(OB):
                lns = small.tile([128, 1], F32, tag="lns", name=f"lns{tt}")
                nc.scalar.activation(lns[:], sall[:, tt: tt + 1], AF.Ln)
                nc.vector.tensor_scalar(final[:, tt * C: (tt + 1) * C],
                                        e1all[:, tt * C: (tt + 1) * C],
                                        lns[:], None, mybir.AluOpType.subtract)
            nc.sync.dma_start(
                out=d_out[:].rearrange("(b p) c -> p b c", p=128),
                in_=final[:])

    nc.compile()
    return nc


def _pack_inputs(x, adj, W, a, W_out, a_out, n_cores=N_CORES):
    """Host-side shard + layout packing. Returns list of per-core in_maps."""
    n, f_in = x.shape
    OWN = n // n_cores
    NB = n // 128
    xf = np.asarray(x, np.float32)
    Wf = np.asarray(W, np.float32)
    af = np.asarray(a, np.float32)
    Wof = np.asarray(W_out, np.float32)
    aof = np.asarray(a_out, np.float32)

    # xT[p, k*n + m] = x[m, 128k+p]
    xT = xf.T.reshape(KF, 128, n).transpose(1, 0, 2).reshape(128, KF * n)
    xT = xT.astype(BF)
    w64 = (Wf.reshape(H, KF, 128, O).transpose(2, 0, 1, 3)
           .reshape(128, H * KF * O).astype(BF))
    wa1 = np.einsum("hfo,ho->hf", Wf, af[:, :O])  # [H, F]
    wa2 = np.einsum("hfo,ho->hf", Wf, af[:, O:])
    # waA[p, k*16 + m]: m<8 -> wa2[m], else wa1[m-8]
    waA = np.concatenate([wa2, wa1], axis=0)  # [16, F]
    waA = waA.T.reshape(KF, 128, 16).transpose(1, 0, 2).reshape(128, KF * 16)
    waA = waA.astype(BF)
    wo1 = Wof @ aof[:C]  # [F]
    wo1r = np.broadcast_to(
        wo1.reshape(KF, 128).T[:, :, None], (128, KF, 128)
    ).reshape(128, KF * 128).astype(BF)
    wot = (Wof.reshape(KF, 128, C).transpose(1, 0, 2)
           .reshape(128, KF * C).astype(BF))
    a2o = np.broadcast_to(aof[C:], (128, C)).astype(np.float32).copy()

    in_maps = []
    for core in range(n_cores):
        rows = slice(core * OWN, (core + 1) * OWN)
        xo = (xf[rows].T.reshape(KF, 128, OWN).transpose(1, 0, 2)
              .reshape(128, KF * OWN).astype(BF))
        nmT = np.where(adj[rows].T > 0, np.float32(0), np.float32(NEG))
        nmT = (nmT.reshape(NB, 128, OWN).transpose(1, 0, 2)
               .reshape(128, NB * OWN).astype(BF))
        in_maps.append({
            "xT": xT, "xo": xo, "w64": w64, "waA": waA, "wo1r": wo1r,
            "nmT": nmT, "wot": wot, "a2o": a2o,
        })
    return in_maps


_NC_CACHE = {}


def _get_nc(n_cores=N_CORES, n=N):
    key = (n_cores, n)
    if key not in _NC_CACHE:
        _NC_CACHE[key] = _build_nc(n_cores, n)
    return _NC_CACHE[key]


def kernel(x, adj, W, a, W_out, a_out):
    nc = _get_nc()
    in_maps = _pack_inputs(x, adj, W, a, W_out, a_out)
    res = run_bass_kernel_spmd(nc, in_maps, list(range(N_CORES)))
    out = np.concatenate([res.results[c]["out"] for c in range(N_CORES)], axis=0)
    return out.astype(np.float32)
